# revision 15
# baseline (speedup 1.0000x reference)
"""Trainium2 Bass kernel for nn_DistanceTransform (16,1,128,128 f32).

The reference runs n_iters = ceil(128/1) = 128 iterations of
    cdt      = -h * log(conv3x3_replicate(boundary))
    mask     = cdt > 0
    out     += (i*3//2 + cdt) * mask
    boundary = where(mask, 1, boundary)
starting from boundary = image.

For any input with values in (0,1) the masks are identically zero from
iteration 1 onward: `where` only raises boundary values to 1 (monotone
non-decreasing), conv is monotone in boundary, so a pixel with mask=1 at
iter 0 has conv = 1 + (positive neighbor sum) > 1 at iter 1, and a pixel
with mask=0 already had conv >= 1 which cannot decrease.  Every
iteration >= 1 therefore contributes an exact 0.0 and leaves boundary
unchanged, so

    out = relu(-h * log(conv3x3_replicate(image)))     (exactly)

which is what this kernel computes in a single memory-bound pass.

Sharding: pure data parallelism, 2 images per NeuronCore across 8 cores.

Device layout per core: H=128 on partitions; free dim = (2 images x 130
W-padded cols).  The host pads H/W with replicate edges (pure data
movement), the device does all arithmetic:
  2 DMA loads with overlapping-window APs giving three row-shifted
    copies of the padded input (x_up/x_down first — the first DVE op's
    inputs — then x center, needed one op later)
  5 VectorE ops implementing the 9-point conv as
      t  = xu + xd
      w  = (c/b)*t + x
      sw = w<<1 + w>>1          (W-shift sum, replicate via padding)
      r  = (b - c/b)*t_c + w_c
      y  = b*sw + r             ( = x + b*(xl+xr+xu+xd) + c*corners )
  1 ScalarE op: l = Ln(y)
  1 VectorE op: out = max(-h*l, 0)   (fp32 tensor_scalar runs 2x)
  1 DMA store.
The Ln bias is an explicit tile memset on the idle DVE so the framework
emits no const-AP memsets on the preamble path; the dead framework const
memsets are then stripped (-250ns off the pre-DMA barrier).

All arithmetic is exact fp32.  Alternatives measured on HW and rejected:
PE float32r matmuls are fast (1 cyc/row) but round inputs to ~tf32
(rel err 8.8e-5 vs 6.7e-6); PE fp32 matmuls are bit-exact-class but
their model win rests on un-modeled HAM warmup + fused weight-load cost.
TimelineSim cost-model device time: ~9.8 us/core, dominated by fixed
per-DMA latencies (SEQ+HWDGE+DGE ~1.8us, sem-propagation 0.9us each for
input and output) and the framework preamble/exit barriers; the actual
bytes are only ~1.1 us.
"""

import numpy as np

H_PARAM = 0.35
B_FULL = 16
IMG = 128
N_CORES = 8
B_LOC = B_FULL // N_CORES  # 2

_CACHE = {}


def _coeffs():
    # match the reference's fp32 kernel construction bit-for-bit:
    # dist = hypot(dx,dy) in f32; weight = exp(-dist/h) in f32
    h = np.float32(H_PARAM)
    b = np.exp(np.float32(-1.0) / h).astype(np.float32)
    c = np.exp(-np.hypot(np.float32(1.0), np.float32(1.0)) / h).astype(np.float32)
    alpha = np.float32(np.float64(c) / np.float64(b))
    beta = np.float32(np.float64(b) - np.float64(alpha))
    return float(b), float(alpha), float(beta)


def _legalize_single_wait(nc):
    """This walrus encodes at most ONE sync-wait per instruction.  Tile can
    attach several (e.g. the kernel-tail drain).  Split extras onto NoOps
    inserted just before the offending instruction on the same engine."""
    import concourse.mybir as mybir

    n = 0
    for bb in nc.main_func.blocks:
        insts = bb.instructions
        i = 0
        while i < len(insts):
            ins = insts[i]
            si = ins.sync_info
            if si is not None and len(si.on_wait) > 1:
                waits = list(si.on_wait)
                nops = []
                for k, wt in enumerate(waits[:-1]):
                    nop = mybir.InstNoOp(
                        name=f"{ins.name}-w{k}",
                        engine=ins.engine,
                        ins=[],
                        outs=[],
                        sync_info=mybir.SyncInfo(on_wait=[wt], on_update=[]),
                    )
                    nc.register_instruction(nop)
                    nops.append(nop)
                ins.sync_info = mybir.SyncInfo(
                    on_wait=[waits[-1]], on_update=si.on_update
                )
                for nop in reversed(nops):
                    insts.insert(i, nop)
                i += len(nops)
                n += 1
            i += 1
    return n


def _drop_dead_const_memsets(nc):
    """The framework preamble memsets const-AP tensors on Pool before the
    all-engine barrier; with an explicit activation bias none of them have
    readers, and they gate the barrier (~250ns).  Drop memsets whose target
    tensor is never read."""
    read_names = set()
    for bb in nc.main_func.blocks:
        for ins in bb.instructions:
            for a in ins.ins:
                for attr in ("bass_ap", None):
                    try:
                        name = (
                            a.bass_ap.tensor.name if attr else a.memref
                        )
                        read_names.add(name)
                    except Exception:
                        pass
    n = 0
    for bb in nc.main_func.blocks:
        keep = []
        for ins in bb.instructions:
            if type(ins).__name__ == "InstMemset":
                tgt = None
                a = ins.outs[0]
                try:
                    tgt = a.bass_ap.tensor.name
                except Exception:
                    try:
                        tgt = a.memref
                    except Exception:
                        pass
                if (
                    tgt is not None
                    and tgt.startswith("const-")
                    and tgt not in read_names
                    and not (ins.sync_info and (ins.sync_info.on_wait or ins.sync_info.on_update))
                ):
                    n += 1
                    continue
            keep.append(ins)
        if len(keep) != len(bb.instructions):
            bb.instructions[:] = keep
    return n


def _build_nc():
    import concourse.bass as bass
    import concourse.mybir as mybir
    from concourse import tile

    f32 = mybir.dt.float32
    add = mybir.AluOpType.add
    mult = mybir.AluOpType.mult
    mx = mybir.AluOpType.max
    AF = mybir.ActivationFunctionType

    b, alpha, beta = _coeffs()

    nc = bass.Bass(trn_type="TRN2")
    xin = nc.dram_tensor("x", [IMG + 2, B_LOC, IMG + 2], f32, kind="ExternalInput")
    yout = nc.dram_tensor("y", [IMG, B_LOC, IMG], f32, kind="ExternalOutput")

    from concourse.ap import AP

    W2 = IMG + 2
    R = B_LOC * W2
    # Overlapping-window APs: partition p reads padded-input rows {p, p+2}
    # (the two row-shifted copies the first DVE op needs) in DMA 1, and row
    # p+1 (the center copy, needed one op later) in DMA 2 — so the compute
    # chain starts as soon as the smaller first transfer lands.
    src_ud = AP(xin[:].tensor, 0, [[R, IMG], [2 * R, 2], [W2, B_LOC], [1, W2]])
    src_c = AP(xin[:].tensor, R, [[R, IMG], [W2, B_LOC], [1, W2]])

    with tile.TileContext(nc) as tc:
        with tc.tile_pool(name="p", bufs=1) as pool:
            xall = pool.tile([IMG, 3, B_LOC, W2], f32, name="xall")
            t = pool.tile([IMG, B_LOC, W2], f32, name="t")
            w = pool.tile([IMG, B_LOC, W2], f32, name="w")
            sw = pool.tile([IMG, B_LOC, IMG], f32, name="sw")
            r = pool.tile([IMG, B_LOC, IMG], f32, name="r")
            yv = pool.tile([IMG, B_LOC, IMG], f32, name="yv")
            lt = pool.tile([IMG, B_LOC, IMG], f32, name="lt")
            ot = pool.tile([IMG, B_LOC, IMG], f32, name="ot")
            zb = pool.tile([IMG, 1], f32, name="zb")

            nc.sync.dma_start(xall[:, 0::2], src_ud)
            nc.sync.dma_start(xall[:, 1], src_c)

            # explicit Ln bias (zeros) memset on the idle DVE, so the
            # framework doesn't emit a const-AP memset on the preamble path
            nc.vector.memset(zb[:], 0.0)

            xd, xp, xu = xall[:, 0], xall[:, 1], xall[:, 2]
            v = nc.vector
            v.tensor_add(t[:], xu, xd)
            v.scalar_tensor_tensor(w[:], t[:], alpha, xp, op0=mult, op1=add)
            v.tensor_add(sw[:], w[:, :, 0:IMG], w[:, :, 2:W2])
            v.scalar_tensor_tensor(
                r[:], t[:, :, 1 : IMG + 1], beta, w[:, :, 1 : IMG + 1],
                op0=mult, op1=add,
            )
            v.scalar_tensor_tensor(yv[:], sw[:], b, r[:], op0=mult, op1=add)

            nc.scalar.activation(lt[:], yv[:], AF.Ln, bias=zb[:])
            # out = relu(-h * ln(y)) on DVE (tensor_scalar runs 2x for fp32,
            # and this skips a second serial ACT op + sequencer gap)
            v.tensor_scalar(ot[:], lt[:], -H_PARAM, 0.0, op0=mult, op1=mx)

            nc.sync.dma_start(yout[:], ot[:])

    _drop_dead_const_memsets(nc)
    _legalize_single_wait(nc)
    return nc


def get_nc():
    nc = _CACHE.get("nc")
    if nc is None:
        nc = _build_nc()
        _CACHE["nc"] = nc
    return nc


def make_in_maps(image):
    """(16,1,128,128) -> list of 8 per-core dicts with 'x': (130,2,130)."""
    img = np.asarray(image, dtype=np.float32).reshape(B_FULL, IMG, IMG)
    pad = np.pad(img, ((0, 0), (1, 1), (1, 1)), mode="edge")  # (16,130,130)
    in_maps = []
    for i in range(N_CORES):
        shard = pad[i * B_LOC : (i + 1) * B_LOC]  # (2,130,130)
        in_maps.append({"x": np.ascontiguousarray(shard.transpose(1, 0, 2))})
    return in_maps


def assemble(results):
    """list of 8 per-core {'y': (128,2,128)} -> (16,1,128,128)."""
    outs = []
    for i in range(N_CORES):
        y = np.asarray(results[i]["y"])  # (128, B_LOC, 128)
        outs.append(np.ascontiguousarray(y.transpose(1, 0, 2)))
    out = np.concatenate(outs, axis=0).reshape(B_FULL, 1, IMG, IMG)
    return out.astype(np.float32, copy=False)


def kernel(image):
    from concourse.bass_utils import run_bass_kernel_spmd

    nc = get_nc()
    in_maps = make_in_maps(image)
    try:
        res = run_bass_kernel_spmd(nc, in_maps, list(range(N_CORES)))
    except Exception:
        # One retry: a previously wedged NeuronCore (NRT_EXEC_UNIT_UNRECOVERABLE
        # etc.) usually recovers on the next attempt.
        _CACHE.clear()
        nc = get_nc()
        res = run_bass_kernel_spmd(nc, in_maps, list(range(N_CORES)))
    return assemble(res.results)
